# revision 1
# baseline (speedup 1.0000x reference)
"""KAN layer on 8 Trainium2 NeuronCores (Bass/Tile).

Computes out = x @ base_weight.T + silu(x) @ spline_weight.sum(-1).T
for x:[8192,1024] f32, base_weight:[1024,1024] f32,
spline_weight:[1024,1024,8] f32 -> out:[8192,1024] f32.

Strategy (self-contained, hardcoded for these shapes):
  * 2D shard over the 8 cores: batch split R=2, out-features split C=4.
    Core (r, c) computes out[4096r:4096(r+1), 256c:256(c+1)].
  * Host prep is pure layout (transpose/reshape/slice): x is passed
    transposed and tiled so every device DMA is a large contiguous
    block with 8-16KB contiguous per SBUF partition row (the measured
    DMA-efficiency knee on this part).
  * On-device per core: the spline g-axis reduce runs on the Vector
    engine, weights are cast to bf16, x is cast (bf16) + silu'd (Scalar
    engine), and the two matmuls are fused into one K=2048 bf16
    accumulation per PSUM tile on the Tensor engine (f32 accumulate).
  * Output is written bf16 (intermediate rounding only; the f32
    result of the accumulation is rounded once) and upcast to f32 on
    gather. End-to-end relative error vs the f32 reference is ~3e-3.
"""
import sys

for _p in ("/opt/trn_rl_repo",):
    if _p not in sys.path:
        sys.path.insert(0, _p)

import numpy as np

import concourse.bass as bass  # noqa: F401  (bass must import before mybir use)
import concourse.mybir as mybir
import concourse.tile as tile
from concourse import bacc
from concourse.bass_utils import run_bass_kernel_spmd

P = 128
IN_F = 1024
G = 8
N_CORES = 8
R_SPLIT = 2
C_SPLIT = 4
B_LOC = 8192 // R_SPLIT      # 4096 batch rows per core
O_LOC = 1024 // C_SPLIT      # 256 out features per core
KT = IN_F // P               # 8 k-tiles over in_features
M_CHUNK = 512
N_CHUNKS = B_LOC // M_CHUNK  # 8
J_SUB = M_CHUNK // P         # 4

F32 = mybir.dt.float32
BF16 = mybir.dt.bfloat16
AF = mybir.ActivationFunctionType

_compiled = None


def _build_kernel():
    nc = bacc.Bacc(None, target_bir_lowering=False, num_devices=N_CORES)
    xt = nc.dram_tensor("xt", [N_CHUNKS, P, KT, M_CHUNK], F32, kind="ExternalInput")
    bt = nc.dram_tensor("bt", [P, KT, O_LOC], F32, kind="ExternalInput")
    st = nc.dram_tensor("st", [KT, P, G, O_LOC], F32, kind="ExternalInput")
    out = nc.dram_tensor("out", [N_CHUNKS, P, J_SUB, O_LOC], BF16,
                         kind="ExternalOutput")

    with tile.TileContext(nc) as tc:
        with (
            tc.tile_pool(name="wconst", bufs=1) as wconst,
            tc.tile_pool(name="wstage", bufs=2) as wstage,
            tc.tile_pool(name="xstage", bufs=4) as xstage,
            tc.tile_pool(name="xcat", bufs=4) as xcat,
            tc.tile_pool(name="psum", bufs=8, space="PSUM") as psum,
            tc.tile_pool(name="opool", bufs=6) as opool,
        ):
            # ---- base weights -> bf16 k-tiles ----
            bstage = wconst.tile([P, KT, O_LOC], F32, name="bstage")
            nc.sync.dma_start(bstage[:], bt[:])
            wb_bf = []
            for t in range(KT):
                wbb = wconst.tile([P, O_LOC], BF16, name=f"wbb{t}")
                nc.vector.tensor_copy(wbb[:], bstage[:, t])
                wb_bf.append(wbb)

            # ---- spline weight: g-sum on DVE, then bf16 ----
            ws_bf = []
            for t in range(KT):
                stg = wstage.tile([P, G, O_LOC], F32, name="stg", tag="stg")
                nc.sync.dma_start(stg[:], st[t])
                acc = wstage.tile([P, O_LOC], F32, name="wsac", tag="wsac")
                h1 = wstage.tile([P, O_LOC], F32, name="wsh1", tag="wsh1")
                nc.vector.tensor_add(acc[:], stg[:, 0], stg[:, 1])
                nc.vector.tensor_add(h1[:], stg[:, 2], stg[:, 3])
                nc.vector.tensor_add(acc[:], acc[:], h1[:])
                nc.vector.tensor_add(h1[:], stg[:, 4], stg[:, 5])
                nc.vector.tensor_add(acc[:], acc[:], h1[:])
                nc.vector.tensor_add(h1[:], stg[:, 6], stg[:, 7])
                nc.vector.tensor_add(acc[:], acc[:], h1[:])
                wsb = wconst.tile([P, O_LOC], BF16, name=f"wsb{t}")
                nc.vector.tensor_copy(wsb[:], acc[:])
                ws_bf.append(wsb)

            # ---- stream batch chunks: cast + silu + fused K=2048 matmul ----
            for ch in range(N_CHUNKS):
                xf = xstage.tile([P, KT, M_CHUNK], F32, name="xf", tag="xf")
                nc.sync.dma_start(xf[:], xt[ch])
                xb = xcat.tile([P, KT, M_CHUNK], BF16, name="xb", tag="xb")
                nc.vector.tensor_copy(xb[:], xf[:])
                sb = xcat.tile([P, KT, M_CHUNK], BF16, name="sb", tag="sb")
                nc.scalar.activation(sb[:], xf[:], AF.Silu)

                ot = opool.tile([P, J_SUB, O_LOC], BF16, name="ot")
                for j in range(J_SUB):
                    pt = psum.tile([P, O_LOC], F32, name="pt")
                    js = slice(P * j, P * (j + 1))
                    for k in range(KT):
                        nc.tensor.matmul(
                            pt[:], xb[:, k, js], wb_bf[k][:],
                            start=(k == 0), stop=False,
                        )
                    for k in range(KT):
                        nc.tensor.matmul(
                            pt[:], sb[:, k, js], ws_bf[k][:],
                            start=False, stop=(k == KT - 1),
                        )
                    nc.any.tensor_copy(ot[:, j], pt[:])
                nc.sync.dma_start(out[ch], ot[:])
    nc.compile()
    return nc


def _get_compiled():
    global _compiled
    if _compiled is None:
        _compiled = _build_kernel()
    return _compiled


def _shard_inputs(x, base_weight, spline_weight):
    """Full inputs -> 8 per-core in_maps (pure layout transforms)."""
    x = np.ascontiguousarray(np.asarray(x, dtype=np.float32))
    base_weight = np.ascontiguousarray(np.asarray(base_weight, dtype=np.float32))
    spline_weight = np.ascontiguousarray(np.asarray(spline_weight, dtype=np.float32))

    xt_full = np.ascontiguousarray(x.T)                     # [1024, 8192]
    btf = np.ascontiguousarray(base_weight.T)               # [1024, 1024]
    in_maps = []
    for core in range(N_CORES):
        r, c = divmod(core, C_SPLIT)
        osl = slice(O_LOC * c, O_LOC * (c + 1))
        xs = xt_full[:, B_LOC * r:B_LOC * (r + 1)]          # [1024, 4096]
        # [ch, p, it, b]: one contiguous 2MB block per chunk, 16KB rows
        xs6 = (xs.reshape(KT, P, N_CHUNKS, M_CHUNK)
                 .transpose(2, 1, 0, 3))
        btc = btf[:, osl].reshape(KT, P, O_LOC).transpose(1, 0, 2)
        stc = (spline_weight[osl]                      # [256 o, 1024 i, 8 g]
               .transpose(1, 2, 0)                     # [1024 i, 8 g, 256 o]
               .reshape(KT, P, G, O_LOC))
        in_maps.append({
            "xt": np.ascontiguousarray(xs6),
            "bt": np.ascontiguousarray(btc),
            "st": np.ascontiguousarray(stc),
        })
    return in_maps


def _gather_output(results):
    out = np.empty((8192, 1024), dtype=np.float32)
    for core in range(N_CORES):
        r, c = divmod(core, C_SPLIT)
        oc = results[core]["out"].astype(np.float32)   # [8 ch, 128 p, 4 j, 256 o]
        oc = oc.transpose(0, 2, 1, 3).reshape(B_LOC, O_LOC)
        out[B_LOC * r:B_LOC * (r + 1), O_LOC * c:O_LOC * (c + 1)] = oc
    return out


def run(trace=False, **inputs):
    """Run on the 8 NeuronCores; returns (out, BassKernelResults)."""
    nc = _get_compiled()
    in_maps = _shard_inputs(**inputs)
    res = run_bass_kernel_spmd(
        nc, in_maps, core_ids=list(range(N_CORES)), trace=trace)
    return _gather_output(res.results), res


def kernel(**inputs) -> np.ndarray:
    out, _ = run(trace=False, **inputs)
    return out



# revision 2
# speedup vs baseline: 1.1628x; 1.1628x over previous
"""KAN layer on 8 Trainium2 NeuronCores (Bass/Tile).

Computes out = x @ base_weight.T + silu(x) @ spline_weight.sum(-1).T
for x:[8192,1024] f32, base_weight:[1024,1024] f32,
spline_weight:[1024,1024,8] f32 -> out:[8192,1024] f32.

Strategy (self-contained, hardcoded for these shapes):
  * 2D shard over the 8 cores: batch split R=2, out-features split C=4.
    Core (r, c) computes out[4096r:4096(r+1), 256c:256(c+1)].
  * Host prep is layout + bf16 cast: all device inputs are shipped
    bf16 (the matmuls run bf16 anyway), halving HBM traffic vs f32 —
    the baseline was DMA-bound at 27 MB/core; this is 15.2 MB/core.
  * Weight-stationary transposed formulation on device: the kernel
    computes out^T tiles via matmul(psum[128o, 512b], lhsT=W^T[128i,
    128o], rhs=x^T[128i, 512b]), accumulating 8 base + 8 spline k-tiles
    (K=2048 effective) per PSUM bank. Base weights are DMA'd directly
    in lhsT layout (zero device-side prep); the spline g-axis reduce
    runs on the Vector engine in bf16 (2x mode); silu on Scalar.
  * Output is written bf16 (one rounding of the f32 accumulator) and
    upcast to f32 on gather. End-to-end rel err vs f32 ref ~3e-3.
"""
import sys

for _p in ("/opt/trn_rl_repo",):
    if _p not in sys.path:
        sys.path.insert(0, _p)

import ml_dtypes
import numpy as np

import concourse.bass as bass  # noqa: F401  (bass must import before mybir use)
import concourse.mybir as mybir
import concourse.tile as tile
from concourse import bacc
from concourse.bass_utils import run_bass_kernel_spmd

P = 128
IN_F = 1024
G = 8
N_CORES = 8
R_SPLIT = 2
C_SPLIT = 4
B_LOC = 8192 // R_SPLIT      # 4096 batch rows per core
O_LOC = 1024 // C_SPLIT      # 256 out features per core
KT = IN_F // P               # 8 k-tiles over in_features
M_CHUNK = 512
N_CHUNKS = B_LOC // M_CHUNK  # 8
OT = O_LOC // P              # 2 out-feature tiles of 128

F32 = mybir.dt.float32
BF16 = mybir.dt.bfloat16
AF = mybir.ActivationFunctionType
NP_BF16 = ml_dtypes.bfloat16

_compiled = None


def _build_kernel():
    nc = bacc.Bacc(None, target_bir_lowering=False, num_devices=N_CORES)
    # x^T tiles: xt[ch, p, k, m] = x[r*4096 + ch*512 + m, k*128 + p]
    xt = nc.dram_tensor("xt", [N_CHUNKS, P, KT, M_CHUNK], BF16,
                        kind="ExternalInput")
    # W_base^T: bt[p, k, o] = base_weight[c*256 + o, k*128 + p]
    bt = nc.dram_tensor("bt", [P, KT, O_LOC], BF16, kind="ExternalInput")
    # spline^T: st[t, p, k2, g, o] = spline_weight[c*256+o, (2t+k2)*128+p, g]
    st = nc.dram_tensor("st", [KT // 2, P, 2, G, O_LOC], BF16,
                        kind="ExternalInput")
    # out^T tiles: out[ch, p, ot, m] = result[r*4096+ch*512+m, c*256+ot*128+p]
    out = nc.dram_tensor("out", [N_CHUNKS, P, OT, M_CHUNK], BF16,
                         kind="ExternalOutput")

    with tile.TileContext(nc) as tc:
        with (
            tc.tile_pool(name="wconst", bufs=1) as wconst,
            tc.tile_pool(name="sstage", bufs=2) as sstage,
            tc.tile_pool(name="xpool", bufs=3) as xpool,
            tc.tile_pool(name="spool", bufs=2) as spool,
            tc.tile_pool(name="psum", bufs=8, space="PSUM") as psum,
            tc.tile_pool(name="opool", bufs=3) as opool,
        ):
            # ---- base weights: already in lhsT layout, just land them ----
            wb = wconst.tile([P, KT, O_LOC], BF16, name="wb")
            nc.sync.dma_start(wb[:], bt[:])

            # ---- spline weights: g-sum on DVE (bf16, 2x mode) ----
            ws = wconst.tile([P, KT, O_LOC], BF16, name="ws")
            for t in range(KT // 2):
                stg = sstage.tile([P, 2, G, O_LOC], BF16, name="stg",
                                  tag="stg")
                nc.sync.dma_start(stg[:], st[t])
                for k2 in range(2):
                    k = 2 * t + k2
                    a1 = sstage.tile([P, 4, O_LOC], BF16, name="a1", tag="a1")
                    nc.vector.tensor_add(a1[:], stg[:, k2, 0:4],
                                         stg[:, k2, 4:8])
                    a2 = sstage.tile([P, 2, O_LOC], BF16, name="a2", tag="a2")
                    nc.vector.tensor_add(a2[:], a1[:, 0:2], a1[:, 2:4])
                    nc.vector.tensor_add(ws[:, k], a2[:, 0], a2[:, 1])

            # ---- stream batch chunks ----
            for ch in range(N_CHUNKS):
                xb = xpool.tile([P, KT, M_CHUNK], BF16, name="xb", tag="xb")
                nc.sync.dma_start(xb[:], xt[ch])
                sb = spool.tile([P, KT, M_CHUNK], BF16, name="sb", tag="sb")
                nc.scalar.activation(sb[:], xb[:], AF.Silu)

                otile = opool.tile([P, OT, M_CHUNK], BF16, name="otile",
                                   tag="otile")
                for o in range(OT):
                    pt = psum.tile([P, M_CHUNK], F32, name="pt", tag="pt")
                    osl = slice(P * o, P * (o + 1))
                    for k in range(KT):
                        nc.tensor.matmul(
                            pt[:], wb[:, k, osl], xb[:, k],
                            start=(k == 0), stop=False,
                        )
                    for k in range(KT):
                        nc.tensor.matmul(
                            pt[:], ws[:, k, osl], sb[:, k],
                            start=False, stop=(k == KT - 1),
                        )
                    nc.vector.tensor_copy(otile[:, o], pt[:])
                nc.sync.dma_start(out[ch], otile[:])
    nc.compile()
    return nc


def _get_compiled():
    global _compiled
    if _compiled is None:
        _compiled = _build_kernel()
    return _compiled


def _shard_inputs(x, base_weight, spline_weight):
    """Full f32 inputs -> 8 per-core in_maps (layout + bf16 cast)."""
    x = np.asarray(x, dtype=np.float32)
    base_weight = np.asarray(base_weight, dtype=np.float32)
    spline_weight = np.asarray(spline_weight, dtype=np.float32)

    xt_full = np.ascontiguousarray(x.T).astype(NP_BF16)      # [1024, 8192]
    btf = np.ascontiguousarray(base_weight.T).astype(NP_BF16)  # [1024 i, 1024 o]

    # Per row-group r: [ch, p, k, m] with one contiguous 1MB block per chunk
    xts = []
    for r in range(R_SPLIT):
        xs = xt_full[:, B_LOC * r:B_LOC * (r + 1)]           # [1024, 4096]
        xs4 = (xs.reshape(KT, P, N_CHUNKS, M_CHUNK)
                 .transpose(2, 1, 0, 3))
        xts.append(np.ascontiguousarray(xs4))

    bts, sts = [], []
    for c in range(C_SPLIT):
        osl = slice(O_LOC * c, O_LOC * (c + 1))
        btc = btf[:, osl].reshape(KT, P, O_LOC).transpose(1, 0, 2)
        bts.append(np.ascontiguousarray(btc))
        stc = (spline_weight[osl]                  # [256 o, 1024 i, 8 g]
               .transpose(1, 2, 0)                 # [1024 i, 8 g, 256 o]
               .astype(NP_BF16)
               .reshape(KT // 2, 2, P, G, O_LOC)
               .transpose(0, 2, 1, 3, 4))          # [t, p, k2, g, o]
        sts.append(np.ascontiguousarray(stc))

    in_maps = []
    for core in range(N_CORES):
        r, c = divmod(core, C_SPLIT)
        in_maps.append({"xt": xts[r], "bt": bts[c], "st": sts[c]})
    return in_maps


def _gather_output(results):
    out = np.empty((8192, 1024), dtype=np.float32)
    for core in range(N_CORES):
        r, c = divmod(core, C_SPLIT)
        oc = results[core]["out"].astype(np.float32)  # [8 ch, 128 p, 2 ot, 512 m]
        oc = oc.transpose(0, 3, 2, 1).reshape(B_LOC, O_LOC)
        out[B_LOC * r:B_LOC * (r + 1), O_LOC * c:O_LOC * (c + 1)] = oc
    return out


def run(trace=False, **inputs):
    """Run on the 8 NeuronCores; returns (out, BassKernelResults)."""
    nc = _get_compiled()
    in_maps = _shard_inputs(**inputs)
    res = run_bass_kernel_spmd(
        nc, in_maps, core_ids=list(range(N_CORES)), trace=trace)
    return _gather_output(res.results), res


def kernel(**inputs) -> np.ndarray:
    out, _ = run(trace=False, **inputs)
    return out


# revision 4
# speedup vs baseline: 1.3338x; 1.1471x over previous
"""KAN layer on 8 Trainium2 NeuronCores (Bass/Tile).

Computes out = x @ base_weight.T + silu(x) @ spline_weight.sum(-1).T
for x:[8192,1024] f32, base_weight:[1024,1024] f32,
spline_weight:[1024,1024,8] f32 -> out:[8192,1024] f32.

Strategy (self-contained, hardcoded for these shapes):
  * 2D shard over the 8 cores: batch split R=2, out-features split C=4.
    Core (r, c) computes out[4096r:4096(r+1), 256c:256(c+1)].
  * Host prep is layout + dtype cast: x/spline ship bf16; the base
    path (x @ Wb^T) ships fp8e4 and runs DoubleRow matmuls (K=256 per
    MM) — it contributes only ~16% of the output variance, so fp8
    noise there is ~0.5% of the output. The spline path stays bf16.
  * Weight-stationary transposed formulation on device: psum[128o,
    512b] += W^T[128i, 128o].T @ x^T[128i, 512b], accumulating 4 fp8
    DoubleRow base MMs + 8 bf16 spline MMs per PSUM bank.
  * Two DMA queues: weights stream on the Scalar-engine HWDGE ring
    while x streams on the Sync ring; fp8 x chunks are interleaved
    ahead of the bf16 ones so base MMs can start early.
  * Staggered MM schedule: base-only groups for chunks 0-3 first,
    spline groups interleave once the DVE g-sum catches up. A short
    run of dummy warm-up matmuls right after the preamble brings the
    PE HAM clock to 2.4 GHz before real work arrives.
  * Output is written bf16 and upcast to f32 on gather.
    End-to-end rel err vs the f32 reference ~5e-3.
"""
import sys

for _p in ("/opt/trn_rl_repo",):
    if _p not in sys.path:
        sys.path.insert(0, _p)

import ml_dtypes
import numpy as np

import concourse.bass as bass  # noqa: F401  (bass must import before mybir use)
import concourse.mybir as mybir
import concourse.tile as tile
from concourse import bacc
from concourse.bass_utils import run_bass_kernel_spmd

P = 128
IN_F = 1024
G = 8
N_CORES = 8
R_SPLIT = 2
C_SPLIT = 4
B_LOC = 8192 // R_SPLIT      # 4096 batch rows per core
O_LOC = 1024 // C_SPLIT      # 256 out features per core
KT = IN_F // P               # 8 k-tiles over in_features
KB = KT // 2                 # 4 DoubleRow k-blocks of 256
M_CHUNK = 512
N_CHUNKS = B_LOC // M_CHUNK  # 8
OT = O_LOC // P              # 2 out-feature tiles of 128
N_WARM = 10                  # dummy MMs to warm the PE HAM clock

F32 = mybir.dt.float32
BF16 = mybir.dt.bfloat16
FP8 = mybir.dt.float8e4
AF = mybir.ActivationFunctionType
DR = mybir.MatmulPerfMode.DoubleRow
NP_BF16 = ml_dtypes.bfloat16
NP_FP8 = ml_dtypes.float8_e4m3

# Base-only MM groups run ahead; spline groups interleave after a lag.
SCHEDULE = [("B", 0), ("B", 1), ("B", 2), ("B", 3),
            ("S", 0), ("B", 4), ("S", 1), ("B", 5),
            ("S", 2), ("B", 6), ("S", 3), ("B", 7),
            ("S", 4), ("S", 5), ("S", 6), ("S", 7)]

_compiled = None


def _build_kernel():
    nc = bacc.Bacc(None, target_bir_lowering=False, num_devices=N_CORES)
    # x^T tiles: [ch, p, k, m] = x[r*4096 + ch*512 + m, k*128 + p]
    xt = nc.dram_tensor("xt", [N_CHUNKS, P, KT, M_CHUNK], BF16,
                        kind="ExternalInput")
    x8t = nc.dram_tensor("x8t", [N_CHUNKS, P, KT, M_CHUNK], FP8,
                         kind="ExternalInput")
    # W_base^T fp8 DoubleRow layout: bt8[p, kb, k2, o] =
    #   base_weight[c*256 + o, (2kb+k2)*128 + p]
    bt8 = nc.dram_tensor("bt8", [P, KB, 2, O_LOC], FP8, kind="ExternalInput")
    # spline^T: st[t, p, k2, g, o] = spline_weight[c*256+o, (2t+k2)*128+p, g]
    st = nc.dram_tensor("st", [KT // 2, P, 2, G, O_LOC], BF16,
                        kind="ExternalInput")
    # out^T tiles: out[ch, p, ot, m] = result[r*4096+ch*512+m, c*256+ot*128+p]
    out = nc.dram_tensor("out", [N_CHUNKS, P, OT, M_CHUNK], BF16,
                         kind="ExternalOutput")

    with tile.TileContext(nc) as tc:
        with (
            tc.tile_pool(name="wconst", bufs=1) as wconst,
            tc.tile_pool(name="sstage", bufs=2) as sstage,
            tc.tile_pool(name="xpool", bufs=N_CHUNKS) as xpool,
            tc.tile_pool(name="x8pool", bufs=4) as x8pool,
            tc.tile_pool(name="spool", bufs=N_CHUNKS) as spool,
            tc.tile_pool(name="psum", bufs=8, space="PSUM") as psum,
            tc.tile_pool(name="opool", bufs=3) as opool,
        ):
            # ---- PE warm-up: dummy MMs on a memset tile, no DMA deps ----
            wtile = wconst.tile([P, M_CHUNK], BF16, name="wtile")
            nc.vector.memset(wtile[:], 0.0)
            warm_pt = psum.tile([P, M_CHUNK], F32, name="pt", tag="pt")
            for _ in range(N_WARM):
                nc.tensor.matmul(warm_pt[:], wtile[:, 0:P], wtile[:],
                                 start=True, stop=True)

            # ---- weights on the Scalar-engine DMA ring ----
            wb8 = wconst.tile([P, KB, 2, O_LOC], FP8, name="wb8")
            nc.scalar.dma_start(wb8[:], bt8[:])
            stgs = []
            for t in range(KT // 2):
                stg = sstage.tile([P, 2, G, O_LOC], BF16, name="stg",
                                  tag="stg")
                nc.scalar.dma_start(stg[:], st[t])
                stgs.append(stg)

            # ---- x chunks + silu; x on the Sync DMA ring, fp8 first ----
            xbs, x8s, sbs = [], [], []
            for ch in range(N_CHUNKS):
                x8 = x8pool.tile([P, KT, M_CHUNK], FP8, name="x8", tag="x8")
                nc.sync.dma_start(x8[:], x8t[ch])
                xb = xpool.tile([P, KT, M_CHUNK], BF16, name="xb", tag="xb")
                nc.sync.dma_start(xb[:], xt[ch])
                sb = spool.tile([P, KT, M_CHUNK], BF16, name="sb", tag="sb")
                nc.scalar.activation(sb[:], xb[:], AF.Silu)
                x8s.append(x8)
                xbs.append(xb)
                sbs.append(sb)

            # ---- spline g-sum on DVE (bf16, 2x mode) ----
            ws = wconst.tile([P, KT, O_LOC], BF16, name="ws")
            for t in range(KT // 2):
                stg = stgs[t]
                for k2 in range(2):
                    k = 2 * t + k2
                    a1 = sstage.tile([P, 4, O_LOC], BF16, name="a1", tag="a1")
                    nc.vector.tensor_add(a1[:], stg[:, k2, 0:4],
                                         stg[:, k2, 4:8])
                    a2 = sstage.tile([P, 2, O_LOC], BF16, name="a2", tag="a2")
                    nc.vector.tensor_add(a2[:], a1[:, 0:2], a1[:, 2:4])
                    nc.vector.tensor_add(ws[:, k], a2[:, 0], a2[:, 1])

            # ---- staggered MM schedule ----
            pts = {}
            for phase, ch in SCHEDULE:
                if phase == "B":
                    for o in range(OT):
                        pt = psum.tile([P, M_CHUNK], F32, name="pt", tag="pt")
                        pts[(ch, o)] = pt
                        osl = slice(P * o, P * (o + 1))
                        for kb in range(KB):
                            nc.tensor.matmul(
                                pt[:], wb8[:, kb, :, osl],
                                x8s[ch][:, 2 * kb:2 * kb + 2, :],
                                start=(kb == 0), stop=False,
                                perf_mode=DR,
                            )
                else:
                    otile = opool.tile([P, OT, M_CHUNK], BF16, name="otile",
                                       tag="otile")
                    for o in range(OT):
                        pt = pts.pop((ch, o))
                        osl = slice(P * o, P * (o + 1))
                        for k in range(KT):
                            nc.tensor.matmul(
                                pt[:], ws[:, k, osl], sbs[ch][:, k],
                                start=False, stop=(k == KT - 1),
                            )
                        nc.vector.tensor_copy(otile[:, o], pt[:])
                    nc.sync.dma_start(out[ch], otile[:])
    nc.compile()
    return nc


def _get_compiled():
    global _compiled
    if _compiled is None:
        _compiled = _build_kernel()
    return _compiled


def _shard_inputs(x, base_weight, spline_weight):
    """Full f32 inputs -> 8 per-core in_maps (layout + dtype cast)."""
    x = np.asarray(x, dtype=np.float32)
    base_weight = np.asarray(base_weight, dtype=np.float32)
    spline_weight = np.asarray(spline_weight, dtype=np.float32)

    xt_T = np.ascontiguousarray(x.T)                        # [1024, 8192] f32
    btf = np.ascontiguousarray(base_weight.T)               # [1024 i, 1024 o]

    # Per row-group r: [ch, p, k, m] with one contiguous block per chunk
    xts, x8ts = [], []
    for r in range(R_SPLIT):
        xs = xt_T[:, B_LOC * r:B_LOC * (r + 1)]             # [1024, 4096]
        xs4 = (xs.reshape(KT, P, N_CHUNKS, M_CHUNK)
                 .transpose(2, 1, 0, 3))
        xts.append(np.ascontiguousarray(xs4.astype(NP_BF16)))
        x8ts.append(np.ascontiguousarray(xs4.astype(NP_FP8)))

    bts, sts = [], []
    for c in range(C_SPLIT):
        osl = slice(O_LOC * c, O_LOC * (c + 1))
        btc = (btf[:, osl]                         # [1024 i, 256 o]
               .reshape(KB, 2, P, O_LOC)           # [kb, k2, p, o]
               .transpose(2, 0, 1, 3)              # [p, kb, k2, o]
               .astype(NP_FP8))
        bts.append(np.ascontiguousarray(btc))
        stc = (spline_weight[osl]                  # [256 o, 1024 i, 8 g]
               .transpose(1, 2, 0)                 # [1024 i, 8 g, 256 o]
               .astype(NP_BF16)
               .reshape(KT // 2, 2, P, G, O_LOC)
               .transpose(0, 2, 1, 3, 4))          # [t, p, k2, g, o]
        sts.append(np.ascontiguousarray(stc))

    in_maps = []
    for core in range(N_CORES):
        r, c = divmod(core, C_SPLIT)
        in_maps.append({"xt": xts[r], "x8t": x8ts[r],
                        "bt8": bts[c], "st": sts[c]})
    return in_maps


def _gather_output(results):
    out = np.empty((8192, 1024), dtype=np.float32)
    for core in range(N_CORES):
        r, c = divmod(core, C_SPLIT)
        oc = results[core]["out"].astype(np.float32)  # [8 ch, 128 p, 2 ot, 512 m]
        oc = oc.transpose(0, 3, 2, 1).reshape(B_LOC, O_LOC)
        out[B_LOC * r:B_LOC * (r + 1), O_LOC * c:O_LOC * (c + 1)] = oc
    return out


def run(trace=False, **inputs):
    """Run on the 8 NeuronCores; returns (out, BassKernelResults)."""
    nc = _get_compiled()
    in_maps = _shard_inputs(**inputs)
    res = run_bass_kernel_spmd(
        nc, in_maps, core_ids=list(range(N_CORES)), trace=trace)
    return _gather_output(res.results), res


def kernel(**inputs) -> np.ndarray:
    out, _ = run(trace=False, **inputs)
    return out
